# revision 81
# baseline (speedup 1.0000x reference)
"""Multi-head attention (16 heads, N=2048, D=1024, E=64) on 8 Trainium2 cores.

Head-parallel sharding: core m handles heads (2m, 2m+1), computes its two
heads' attention contexts and a partial o_proj (rows 128m:128m+128 of the
row-sharded o_proj); the host sums the 8 partial fp32 outputs in fp64.

Numerics (validated baseline scheme): the softmax-score path is
fp32-accurate via an exact hi/lo float32r decomposition (host RNE-11
rounding == the PE's f32r operand rounding); v/ctx/o_proj tolerate plain
f32r.  Scores per [m=128, q=512] tile are two f32r matmuls: a stacked
K=128 cross-term matmul plus a K=65 hi matmul whose row 64 carries
-rowmax(q) (computed by a separate hi-only max pass in [q, m] layout,
DVE-reduced); exp((S-c)/8) on ACT; ctx^T/Z accumulate on PE via a
ones-column in v_ext; 1/Z broadcast+mul normalizes.

Schedule (v2, restructured):
  - x arrives HOST-split into hi/lo f32r planes [128, DCH, N] (c-major:
    per-c slices are 1KB-contiguous DMAs).  No device-side split work.
  - chunk 0 streams wq/xh as c-pair DMAs and defers the xl-dependent
    projection terms to the end of the accumulation group, so the first
    matmul issues ~4us earlier.
  - q/k hi/lo partition layouts alternate per head (and the k weight is
    head-swapped on host) so the psum->hi round-copy (ACT) and the lo
    subtract (DVE, psum+sbuf operands at equal start partitions) need no
    scratch; the ext-hi copy runs on Pool.  pt_q/pt_k psum tiles are
    double-buffered so the next chunk's matmuls never wait on unpack.
  - o_proj results are DMA'd DIRECTLY from PSUM to DRAM (no SBUF bounce).
  - the max-pass units ride the projection chunks (DVE is otherwise idle
    there): comp0 (m 0:1024) spreads over chunks 3-6, chunk 7 carries
    qc0's comp1 interleaved behind its k unpack, and attention chunks
    0-2 carry the remaining comp1 groups with late finish/staging.
  - the drain is pipelined: the last chunk runs its heads sequentially,
    the final normalize is emitted in 128-col slices, and the last
    o_proj blocks rotate through the score psum banks with per-half
    PSUM->DRAM DMAs on both HWDGE queues.
"""
import sys

sys.path.insert(0, "/opt/trn_rl_repo")

from contextlib import ExitStack

import ml_dtypes
import numpy as np

import concourse.bass as bass
import concourse.mybir as mybir
import concourse.tile as tile
from concourse import bacc
from concourse.bass_utils import run_bass_kernel_spmd
from concourse.masks import make_identity

# problem shapes (hardcoded per contract)
N = 2048
D = 1024
E = 64
H = 16
N_CORES = 8
H_PER_CORE = H // N_CORES  # 2

QC = 512          # q-chunk (moving dim of S'/ctx matmuls)
NQ = N // QC      # 4
MB = 128          # m-block (partition dim of S'^T tiles)
NMB = N // MB     # 16
DCH = D // 128    # 8 d-chunks for projections
PC = 256          # projection n-chunk
NPC = N // PC     # 8

F32 = mybir.dt.float32
F32R = mybir.dt.float32r
BF16 = mybir.dt.bfloat16

_CACHE = {}


def build_nc():
    nc = bacc.Bacc(None, target_bir_lowering=False, debug=False)

    # x^T as TWO bf16 planes (base + residual), c-major: together they
    # carry ~17 mantissa bits of x — bf16xbf16 products are EXACT in the
    # fp32 psum, so q/k keep full accuracy from 3 accumulation terms
    # while x moves HALF the bytes of fp32 and needs NO device split.
    xb = nc.declare_dram_parameter("xb", [128, DCH, N], BF16, isOutput=False)
    xr = nc.declare_dram_parameter("xr", [128, DCH, N], BF16, isOutput=False)
    # q/k weights arrive as bf16 base|residual pairs (cols 0:128 | 128:256);
    # k has the HEADS SWAPPED ([h1|h0]) so each head's hi rows land where
    # kx wants them; v is base-only.
    wq = nc.declare_dram_parameter("wq", [D, 256], BF16, isOutput=False)
    wk = nc.declare_dram_parameter("wk", [D, 256], BF16, isOutput=False)
    wv = nc.declare_dram_parameter("wv", [D, 128], BF16, isOutput=False)
    wo = nc.declare_dram_parameter("wo", [128, D], F32R, isOutput=False)
    # bf16 output: the host sums 8 partials in fp64, and bf16 rounding
    # of the partials costs ~2e-3 relative vs the 2e-2 budget — for HALF
    # the output DMA bytes (the drain flush is DMA-serialized)
    out = nc.declare_dram_parameter("out", [N, D], BF16, isOutput=True)

    with ExitStack() as ctx:
        tc = ctx.enter_context(tile.TileContext(nc))
        singles = ctx.enter_context(tc.tile_pool(name="singles", bufs=1))
        bc_pool = ctx.enter_context(tc.tile_pool(name="bc", bufs=2))

        ident = singles.tile([128, 128], F32)

        # long-lived SBUF tensors
        qT_ext = [singles.tile([65, N], F32R, tag=f"qT_ext{h}", name=f"qT_ext{h}")
                  for h in range(2)]
        kT_ext = [singles.tile([65, N], F32R, tag=f"kT_ext{h}", name=f"kT_ext{h}")
                  for h in range(2)]
        # stacked cross-term operands: one K=128 matmul computes
        # kl@qh + kh@ql.  Per-head partition layouts (hi where the psum
        # rows land, so unpack needs no partition shifts except the ext
        # copies):
        #   qx[0] = [qh; ql]   qx[1] = [ql; qh]
        #   kx[0] = [kl; kh]   kx[1] = [kh; kl]
        qx = [singles.tile([128, N], F32R, tag=f"qx{h}", name=f"qx{h}")
              for h in range(2)]
        kx = [singles.tile([128, N], F32R, tag=f"kx{h}", name=f"kx{h}")
              for h in range(2)]
        v_ext = [singles.tile([128, NMB, 65], F32R, tag=f"v_ext{h}",
                              name=f"v_ext{h}") for h in range(2)]
        mneg = [singles.tile([128, NQ], F32, tag=f"mneg{h}", name=f"mneg{h}")
                for h in range(2)]
        ctxn = singles.tile([128, N], F32R, tag="ctxn")
        wo_sb = singles.tile([128, D], F32R, tag="wo_sb")
        # per-chunk partial maxes: m4[qc][h][p, qbl, pair] (pair = 2 m-chunks)
        m4 = {}

        # hi partition ranges per head: qx hi at the head's own psum rows;
        # kx hi at the OTHER half (k weight is head-swapped on host)
        QHI = [slice(0, 64), slice(64, 128)]
        QLO = [slice(64, 128), slice(0, 64)]
        KHI = [slice(64, 128), slice(0, 64)]
        KLO = [slice(0, 64), slice(64, 128)]

        def mp_unit(pool, qc_t, qbl, mc, h, bufs=2, m0=None, mlen=QC,
                    comp=None):
            # one max-pass unit: one hi-only S[q, m] matmul (512-wide
            # m-chunk by default) into a 1-bank psum tile, one X-reduce
            # -> m4[qc_t][h][:, qbl, comp].
            qb = qc_t * (QC // 128) + qbl
            if m0 is None:
                m0 = mc * QC
            if comp is None:
                comp = mc
            t = pool.tile([128, QC], F32, tag="mp", name="mp", bufs=bufs)
            nc.tensor.matmul(
                t[:, 0:mlen],
                qT_ext[h][0:64, qb * 128:(qb + 1) * 128],
                kT_ext[h][0:64, m0:m0 + mlen],
                start=True,
                stop=True,
            )
            nc.vector.tensor_reduce(
                out=m4[qc_t][h][:, qbl, comp:comp + 1],
                in_=t[:, 0:mlen],
                axis=mybir.AxisListType.X, op=mybir.AluOpType.max,
            )

        # per-generation partial-max tiles (tiny; one per (gen, head) so
        # no pool-rotation hazard can serialize a late finish against an
        # early next-generation unit)
        for g in range(NQ):
            nco = 5 if g == 0 else NQ
            m4[g] = [singles.tile([128, NQ, nco], F32, tag=f"m4_{g}_{h}",
                                  name=f"m4_{g}_{h}") for h in range(2)]

        def mp_finish_reduce(qc):
            for h in range(2):
                # combine the component maxes per q-block, negated
                nc.vector.tensor_reduce(
                    out=mneg[h], in_=m4[qc][h],
                    axis=mybir.AxisListType.X, op=mybir.AluOpType.max,
                    negate=True,
                )

        def mp_finish_stage(qc, ptm_pool, ptm_tag, ptm_shape):
            qsl = slice(qc * QC, (qc + 1) * QC)
            # stage -max into qT_ext row 64: transpose [128, 4] -> [4, 128]
            # (rounded to f32r); the partition-major stream of [4, 128] is
            # exactly [1, 512].
            for h in range(2):
                ptm = ptm_pool.tile(ptm_shape, F32, tag=ptm_tag, name="ptm",
                                    space="PSUM")
                nc.tensor.transpose(ptm[0:4, 0:128], mneg[h], ident)
                mt_sb = bc_pool.tile([4, 128], F32R, tag="mt_sb")
                # ACT copy: keeps the staging chain off the DVE queue
                nc.scalar.copy(out=mt_sb, in_=ptm[0:4, 0:128])
                nc.sync.dma_start(out=qT_ext[h][64:65, qsl], in_=mt_sb)

        # ---------------- phase 1: projections ----------------
        with tc.tile_pool(name="mp1", bufs=2, space="PSUM") as mp1, \
             tc.tile_pool(name="xs", bufs=3) as xs_pool, \
             tc.tile_pool(name="ph1", bufs=1) as ph1:
            ones_cols = ph1.tile([128, NMB, 1], F32)
            ones_row = ph1.tile([1, N], F32)

            w_sb = {
                "q": ph1.tile([128, DCH, 256], BF16, tag="w_q", name="w_q"),
                "k": ph1.tile([128, DCH, 256], BF16, tag="w_k", name="w_k"),
                "v": ph1.tile([128, DCH, 128], BF16, tag="w_v", name="w_v"),
            }
            wq_r = wq.rearrange("(c p) e -> p c e", p=128)
            wk_r = wk.rearrange("(c p) e -> p c e", p=128)
            wv_r = wv.rearrange("(c p) e -> p c e", p=128)

            # max-pass ride schedule: unit (g, qbl, mc, h) is ready at the
            # end of proj chunk max(qb//2, 2mc+1) (q/k unpack of its
            # operand ranges).  mc=3 units depend on chunk 7's k unpack:
            # qc0's interleave chunk 7's q matmuls, the rest run in
            # attention.  Greedy placement in generation order fills the
            # proj chunks up to a DVE-budget capacity; the remainder
            # streams through attention slots with per-generation
            # finish/staging emitted right after the last unit.
            proj_cap = {1: 4, 2: 5, 3: 8, 4: 8, 5: 8, 6: 3, 7: 0}
            ride = {c: [] for c in proj_cap}
            att_rest = []
            units_012 = [(g, qbl, mc, h)
                         for g in range(4) for mc in range(3)
                         for qbl in range(4) for h in range(2)]
            for u in sorted(units_012,
                            key=lambda u: (u[0], u[2], u[1], u[3])):
                g, qbl, mc, h = u
                r = max((g * 4 + qbl) // 2, 2 * mc + 1)
                for c in range(max(r, 1), NPC):
                    if len(ride[c]) < proj_cap[c]:
                        ride[c].append(u)
                        break
                else:
                    att_rest.append(u)
            # attention stream: gen1 mc3 first (earliest deadline), then
            # per-generation leftovers + mc3 units
            att_stream = []
            for g in range(1, 4):
                att_stream += [(g, qbl, 3, h)
                               for qbl in range(4) for h in range(2)]
                att_stream += [u for u in att_rest if u[0] == g]
            # sanity: nothing from gen0 may remain
            assert not [u for u in att_rest if u[0] == 0]
            # assign to (chunk, t) slots, inserting finish/staging after
            # each generation completes; staging(g) must land in an
            # attention chunk < g.  Chunk 0 only takes even slots (its
            # entry also absorbs the post-proj DVE backlog).
            # chunk 0 takes sparse slots starting at t=6 (its entry also
            # absorbs the chunk-7/post-proj DVE backlog), denser later;
            # chunks 1-2 dense
            slot_list = ([(0, t) for t in range(6, 17, 2)]
                         + [(0, t) for t in range(17, 32)]
                         + [(1, t) for t in range(32)]
                         + [(2, t) for t in range(32)])
            att_mp = {0: {}, 1: {}, 2: {}}
            att_fin = {0: {}, 1: {}, 2: {}}
            slot = 0
            for g in range(1, 4):
                gu = [u for u in att_stream if u[0] == g]
                for u in gu:
                    fc, ft = slot_list[slot]
                    att_mp.setdefault(fc, {}).setdefault(ft, []).append(u)
                    slot += 1
                fc, ft = slot_list[slot]
                assert fc < g, f"gen {g} staging lands in chunk {fc}"
                att_fin.setdefault(fc, {}).setdefault(ft, []).append(g)

            xtiles = {}

            def xtile(c):
                if c not in xtiles:
                    xtiles[c] = (
                        xs_pool.tile([128, DCH, PC], BF16, tag="xbt",
                                     name=f"xbt{c}", bufs=3),
                        xs_pool.tile([128, DCH, PC], BF16, tag="xrt",
                                     name=f"xrt{c}", bufs=3),
                    )
                return xtiles[c]

            def xsl(c):
                return slice(c * PC, (c + 1) * PC)

            with tc.tile_pool(name="pp", bufs=1, space="PSUM") as pp:
                for nchunk in range(NPC):
                    sl = xsl(nchunk)
                    xbt, xrt = xtile(nchunk)
                    if nchunk == 0:
                        # c-pair streaming on a single queue: the first q
                        # matmuls start once wq01+xb01 land (~3.6us);
                        # wk/xr/xb1/wv queue behind.  The xr-dependent
                        # terms are emitted last in each accumulation
                        # group, so only the base plane gates the start.
                        for cs in (slice(0, 2), slice(2, 4), slice(4, 8)):
                            nc.sync.dma_start(out=w_sb["q"][:, cs, :],
                                              in_=wq_r[:, cs, :])
                            nc.sync.dma_start(out=xbt[:, cs, :],
                                              in_=xb[:, cs, sl])
                        nc.sync.dma_start(out=w_sb["k"], in_=wk_r)
                        nc.sync.dma_start(out=xrt, in_=xr[:, :, sl])
                        nxb, nxr = xtile(1)
                        nc.sync.dma_start(out=nxb, in_=xb[:, :, xsl(1)])
                        nc.sync.dma_start(out=w_sb["v"], in_=wv_r)
                        nc.sync.dma_start(out=nxr, in_=xr[:, :, xsl(1)])
                        # constants built here, off the critical queues
                        # (ident feeds chunk 0's v_ext transposes)
                        make_identity(nc, ident)
                        # PE warm-up: the p-state ramp needs ~3us of
                        # continuous execution to reach 2.4GHz; dummy
                        # transposes burn the DMA-latency window so the
                        # first real matmuls run at full speed
                        for wu in range(12):
                            wut = pp.tile([128, PC], F32, tag="pt_v",
                                          name="warm", bufs=2)
                            nc.tensor.transpose(wut[:, 0:128], ident, ident)
                    elif nchunk < NPC - 1:
                        # prefetch the NEXT chunk's x planes
                        nxb, nxr = xtile(nchunk + 1)
                        nc.sync.dma_start(out=nxb,
                                          in_=xb[:, :, xsl(nchunk + 1)])
                        nc.sync.dma_start(out=nxr,
                                          in_=xr[:, :, xsl(nchunk + 1)])
                        if nchunk == 3:
                            # wo is first read at attention qc1 (~85us);
                            # on the SP queue so it cannot be hoisted into
                            # the startup DMA wall
                            nc.sync.dma_start(out=wo_sb, in_=wo[:, :])
                    if nchunk == 1:
                        # ones constants: built here so their memsets can't
                        # be scheduler-hoisted ahead of make_identity on
                        # the Pool queue (the warm-up chain needs ident)
                        nc.gpsimd.memset(ones_cols, 1.0)
                        nc.gpsimd.memset(ones_row, 1.0)
                    if nchunk in (2, 3):
                        # extension constants (kT_ext row 64 = 1, v_ext
                        # col 64 = 1): written mid-proj where ACT has
                        # slack — attention(0)'s first ext/ctx matmuls
                        # read them immediately at the phase boundary
                        h = nchunk - 2
                        nc.scalar.copy(out=kT_ext[h][64:65, :], in_=ones_row)
                        nc.scalar.copy(out=v_ext[h][:, :, 64:65],
                                       in_=ones_cols)
                    ride_units = ride.get(nchunk, [])
                    pt = {
                        name: pp.tile([128, PC], F32, tag=f"pt_{name}",
                                      name=f"pt_{name}", bufs=2)
                        for name in ("q", "k", "v")
                    }

                    def emit8(name, wsl, xt_, start, stop):
                        for c in range(DCH):
                            nc.tensor.matmul(
                                pt[name],
                                w_sb[name][:, c, wsl],
                                xt_[:, c, :],
                                start=(start and c == 0),
                                stop=(stop and c == DCH - 1),
                            )

                    def unpack(name):
                        dst_ext = qT_ext if name == "q" else kT_ext
                        dst_x = qx if name == "q" else kx
                        hi_sl = QHI if name == "q" else KHI
                        lo_sl = QLO if name == "q" else KLO
                        # unpack: ACT round-copies psum -> f32r hi (same
                        # partitions); DVE subtracts (psum, sbuf-hi) ->
                        # f32r lo at the opposite half; Pool copies the
                        # ext hi view (partition shift where needed).
                        for h in range(2):
                            # psum rows for this head: q keeps [h0|h1],
                            # k is head-swapped on host so hs == hi_sl[h]
                            hs = hi_sl[h] if name == "k" else \
                                slice(h * 64, (h + 1) * 64)
                            nc.scalar.copy(
                                out=dst_x[h][hi_sl[h], sl],
                                in_=pt[name][hs, :])
                            nc.vector.tensor_sub(
                                dst_x[h][lo_sl[h], sl],
                                pt[name][hs, :], dst_x[h][hi_sl[h], sl])
                            if hi_sl[h].start == 0:
                                # partition-aligned: Pool takes it
                                nc.gpsimd.tensor_copy(
                                    dst_ext[h][0:64, sl],
                                    dst_x[h][hi_sl[h], sl])
                            else:
                                # partition SHIFT (64:128 -> 0:64): only
                                # ACT is known to support this
                                nc.scalar.copy(
                                    out=dst_ext[h][0:64, sl],
                                    in_=dst_x[h][hi_sl[h], sl])

                    def emit_v():
                        emit8("v", slice(0, 128), xbt, True, True)
                        vT_c = xs_pool.tile([128, PC], F32, tag="vT_c")
                        nc.scalar.copy(out=vT_c, in_=pt["v"])
                        # v_ext: transpose vT [64, 128-block] -> v [m, e]
                        # blocks [128, 64], inline per chunk
                        for nb2 in range(PC // 128):
                            mb = nchunk * (PC // 128) + nb2
                            for h in range(2):
                                ptt = pp.tile([128, PC], F32, tag="pt_v",
                                              name="ptt", bufs=2)
                                nc.tensor.transpose(
                                    ptt[:, 0:64],
                                    vT_c[h * 64:(h + 1) * 64,
                                         nb2 * 128:(nb2 + 1) * 128],
                                    ident[h * 64:(h + 1) * 64,
                                          h * 64:(h + 1) * 64],
                                )
                                nc.scalar.copy(out=v_ext[h][:, mb, 0:64],
                                               in_=ptt[:, 0:64])

                    HIW, LOW = slice(0, 128), slice(128, 256)
                    if nchunk == 0:
                        # q runs all three terms first (wq is first on the
                        # queue); k/v follow as their weights land
                        emit8("q", HIW, xbt, True, False)
                        emit8("q", LOW, xbt, False, False)
                        emit8("q", HIW, xrt, False, True)
                        unpack("q")
                        emit8("k", HIW, xbt, True, False)
                        emit8("k", LOW, xbt, False, False)
                        emit8("k", HIW, xrt, False, True)
                        unpack("k")
                        emit_v()
                    elif nchunk < NPC - 1:
                        # base-plane terms first, residual terms after,
                        # v last; rides LAST (their operands need this
                        # chunk's unpack)
                        emit8("q", HIW, xbt, True, False)
                        emit8("k", HIW, xbt, True, False)
                        emit8("q", LOW, xbt, False, False)
                        emit8("k", LOW, xbt, False, False)
                        emit8("q", HIW, xrt, False, True)
                        unpack("q")
                        emit8("k", HIW, xrt, False, True)
                        unpack("k")
                        emit_v()
                        for u in ride_units:
                            mp_unit(mp1, *u)
                        if nchunk == NPC - 2:
                            # gen0's mc3a (m 1536:1792) is ready here —
                            # only its second half must wait for chunk 7
                            for u in range(8):
                                mp_unit(mp1, 0, u // 2, 3, u % 2,
                                        m0=1536, mlen=256, comp=3)
                    else:
                        # chunk 7: k completes first; gen0's mc3b units
                        # (m 1792:2048, the only slice still missing)
                        # fire right behind its unpack so the staging
                        # chain completes before attention needs it
                        emit8("k", HIW, xbt, True, False)
                        emit8("k", LOW, xbt, False, False)
                        emit8("k", HIW, xrt, False, True)
                        unpack("k")
                        emit8("q", HIW, xbt, True, False)
                        for u in range(8):
                            mp_unit(mp1, 0, u // 2, 3, u % 2,
                                    m0=1792, mlen=256, comp=4)
                        # gen0's finish + staging fire as soon as its last
                        # unit reduces — ahead of this chunk's q unpack on
                        # the DVE queue, so attention(0) never waits
                        mp_finish_reduce(0)
                        mp_finish_stage(0, mp1, "mp", [128, QC])
                        emit8("q", LOW, xbt, False, False)
                        emit_v()
                        for u in ride_units:
                            mp_unit(mp1, *u)
                        emit8("q", HIW, xrt, False, True)
                        unpack("q")


        # ---------------- phase 2: attention chunks ----------------
        ex_pool = ctx.enter_context(tc.tile_pool(name="ex", bufs=4))
        sp_ps = ctx.enter_context(tc.tile_pool(name="sp", bufs=3, space="PSUM"))
        ctx_pool = ctx.enter_context(tc.tile_pool(name="cx", bufs=1, space="PSUM"))
        mp2 = ctx.enter_context(tc.tile_pool(name="mp2", bufs=1, space="PSUM"))

        def norm_head(qc, h, ctx_ps, sliced=False):
            qsl = slice(qc * QC, (qc + 1) * QC)
            hrows = slice(h * 64, (h + 1) * 64)
            rz = bc_pool.tile([1, QC], F32, tag="rz")
            bc_sb = bc_pool.tile([64, QC], F32, tag="bc_sb")
            if not sliced:
                nc.vector.reciprocal(out=rz, in_=ctx_ps[h][64:65, :])
                nc.gpsimd.partition_broadcast(bc_sb, rz)
                nc.vector.tensor_mul(
                    ctxn[hrows, qsl], ctx_ps[h][0:64, :], bc_sb
                )
                return
            # final-chunk drain: 128-col recip/broadcast/mul chains so the
            # first o_proj block starts ~1us earlier
            for s4 in range(4):
                ssl = slice(s4 * 128, (s4 + 1) * 128)
                nc.vector.reciprocal(out=rz[:, ssl],
                                     in_=ctx_ps[h][64:65, ssl])
                nc.gpsimd.partition_broadcast(bc_sb[:, ssl], rz[:, ssl])
                nc.vector.tensor_mul(
                    ctxn[hrows, qc * QC + s4 * 128:qc * QC + (s4 + 1) * 128],
                    ctx_ps[h][0:64, ssl], bc_sb[:, ssl]
                )

        def oproj_block(qc, nb, fine_dma=False):
            # o_proj for one 128-row n-block (both heads fused: K=128);
            # the two 512-wide psum halves merge into one [128, 1024]
            # SBUF tile so the block is a single contiguous output DMA.
            # The psum->SBUF copies split across DVE (dc=0) and ACT
            # (dc=1) to balance both engines' attention-phase load.
            n0 = qc * QC + nb * 128
            if fine_dma:
                # drain path: no score matmuls left, so the sp 3-bank
                # rotation pipelines the final o_proj matmuls; per-half
                # SBUF tiles (4-deep) and output DMAs alternating between
                # the two HWDGE queues (SP/ACT) keep the drain flowing
                for dc in range(D // QC):
                    pool_, tag_ = (sp_ps, "sp") if (2 * nb + dc) % 2 == 0 \
                        else (mp2, "mp")
                    po = pool_.tile([128, QC], F32, tag=tag_, name="po",
                                    bufs=3)
                    nc.tensor.matmul(
                        po,
                        ctxn[:, n0:n0 + 128],
                        wo_sb[:, dc * QC:(dc + 1) * QC],
                        start=True,
                        stop=True,
                    )
                    pf = ex_pool.tile([128, QC], BF16, tag="po_f", bufs=8)
                    # first half of the flush copies on ACT (DVE still
                    # owns the norm chain), second half on DVE
                    if nb <= 1:
                        nc.scalar.copy(out=pf, in_=po)
                    else:
                        nc.vector.tensor_copy(pf, po)
                    dma_eng = nc.sync if (2 * nb + dc) % 2 == 0 else nc.scalar
                    dma_eng.dma_start(
                        out=out[n0:n0 + 128, dc * QC:(dc + 1) * QC], in_=pf)
                return
            po_sb = ex_pool.tile([128, D], BF16, tag="po_sb", bufs=2)
            for dc in range(D // QC):
                # share the attention "mp" 3-deep psum rotation (frees a
                # dedicated bank and absorbs DVE jitter)
                po = mp2.tile([128, QC], F32, tag="mp", name="po", bufs=3)
                nc.tensor.matmul(
                    po,
                    ctxn[:, n0:n0 + 128],
                    wo_sb[:, dc * QC:(dc + 1) * QC],
                    start=True,
                    stop=True,
                )
                dst = po_sb[:, dc * QC:(dc + 1) * QC]
                if dc == 1 and qc != 2:
                    nc.scalar.copy(out=dst, in_=po)
                else:
                    nc.vector.tensor_copy(dst, po)
            nc.sync.dma_start(out=out[n0:n0 + 128, :], in_=po_sb)

        def attention_chunk(qc, seq_heads=False):
            qsl = slice(qc * QC, (qc + 1) * QC)
            ctx_ps = [ctx_pool.tile([65, QC], F32, tag=f"ctx{h}",
                                    name=f"ctx_ps{h}") for h in range(2)]
            if not seq_heads:
                # h0's ctx matmuls lead (h1's previous-chunk norm frees
                # its bank meanwhile), and h0 finishes ~10 tiles early so
                # its norm chain runs under h1's tail
                heads_order = ([(mb, 0) for mb in range(6)]
                               + [(mb, 1) for mb in range(6)]
                               + [(mb, 0) for mb in range(6, NMB)]
                               + [(mb, 1) for mb in range(6, NMB)])
            else:
                heads_order = [(mb, h) for h in range(2) for mb in range(NMB)]
            started = {0: False, 1: False}

            # interleave schedules: precomputed att_mp/att_fin slot maps
            mp_sched = att_mp.get(qc, {})
            fin_sched = att_fin.get(qc, {})
            po_sched = {}
            if qc > 0:
                for nb in range(4):
                    po_sched[11 + 4 * nb] = (qc - 1, nb)

            def emit_m1_tail(sp, mb, h):
                # the only matmul that reads row 64 (the staged -max row);
                # lagging it one tile behind the cross matmul hides the
                # staging DMA latency at chunk entry
                nc.tensor.matmul(
                    sp, kT_ext[h][:, mb * 128:(mb + 1) * 128],
                    qT_ext[h][:, qsl],
                    start=False, stop=True,
                )
                et = ex_pool.tile([128, QC], F32R, tag="et", name="et")
                nc.scalar.activation(
                    out=et, in_=sp,
                    func=mybir.ActivationFunctionType.Exp, scale=0.125,
                )
                nc.tensor.matmul(
                    ctx_ps[h], v_ext[h][:, mb, :], et,
                    start=not started[h], stop=(mb == NMB - 1),
                )
                started[h] = True
                if mb == NMB - 1 and not seq_heads:
                    # normalize as soon as this head's accumulation closes
                    norm_head(qc, h, ctx_ps)

            lagged = []
            for it, (mb, h) in enumerate(heads_order):
                for g in fin_sched.get(it, ()):
                    mp_finish_reduce(g)
                    mp_finish_stage(g, sp_ps, "sp", [128, QC])
                for u in mp_sched.get(it, ()):
                    mp_unit(mp2, *u, bufs=3)
                if it in po_sched:
                    pqc, pnb = po_sched[it]
                    oproj_block(pqc, pnb)
                msl = slice(mb * 128, (mb + 1) * 128)
                sp = sp_ps.tile([128, QC], F32, tag="sp", name=f"sp{h}")
                # stacked cross terms first (no row-64 dependency):
                # one K=128 matmul = kl@qh + kh@ql
                nc.tensor.matmul(
                    sp, kx[h][:, msl], qx[h][:, qsl],
                    start=True, stop=False,
                )
                lagged.append((sp, mb, h))
                if len(lagged) > 2:
                    emit_m1_tail(*lagged.pop(0))
                if seq_heads and mb == NMB - 1:
                    while lagged:
                        emit_m1_tail(*lagged.pop(0))
                    norm_head(qc, h, ctx_ps, sliced=(h == 1))
            while lagged:
                emit_m1_tail(*lagged.pop(0))
            return ctx_ps

        for qc in range(NQ):
            seq = qc == NQ - 1
            attention_chunk(qc, seq_heads=seq)
        for nb in range(4):
            oproj_block(NQ - 1, nb, fine_dma=True)

    nc.compile()
    return nc


def _round11(x):
    # round-to-nearest-even to 11 explicit mantissa bits — exactly the
    # hardware's float32r operand rounding (verified on device)
    u = np.ascontiguousarray(x, dtype=np.float32).view(np.uint32)
    shift = 23 - 11
    add = np.uint32((1 << (shift - 1)) - 1)
    lsb = (u >> np.uint32(shift)) & np.uint32(1)
    mask = np.uint32(~((1 << shift) - 1) & 0xFFFFFFFF)
    return ((u + add + lsb) & mask).view(np.float32)


def _split11(x):
    hi = _round11(x)
    lo = _round11(x.astype(np.float32) - hi)
    return hi, lo


def _bsplit(a):
    # bf16 base + bf16 residual: ~17 mantissa bits total
    b = a.astype(ml_dtypes.bfloat16)
    r = (a.astype(np.float32) - b.astype(np.float32)).astype(ml_dtypes.bfloat16)
    return b, r


def make_in_map(x, q_proj, k_proj, v_proj, o_proj, core, xf_=None):
    h0 = core * H_PER_CORE
    if xf_ is None:
        xf_ = _x_planes(x)
    xb_, xr_ = xf_

    def wcat2(w, swap):
        pair = [w[h0 + 1], w[h0]] if swap else [w[h0], w[h0 + 1]]
        w2 = np.concatenate(pair, axis=1).astype(np.float32)
        wb, wr = _bsplit(w2)
        return np.ascontiguousarray(np.concatenate([wb, wr], axis=1))

    def wcat(w, swap):
        pair = [w[h0 + 1], w[h0]] if swap else [w[h0], w[h0 + 1]]
        return np.ascontiguousarray(
            np.concatenate(pair, axis=1).astype(np.float32))

    return {
        "xb": xb_,
        "xr": xr_,
        "wq": wcat2(q_proj, False),
        "wk": wcat2(k_proj, True),
        "wv": wcat(v_proj, False).astype(ml_dtypes.bfloat16),
        "wo": _round11(o_proj[h0 * 64:(h0 + 2) * 64, :]),
    }


def _x_planes(x):
    # x^T c-major: [p, c, n] = x[n, 128c+p], split into bf16 base+residual
    xt = np.ascontiguousarray(
        x.astype(np.float32, copy=False).reshape(N, DCH, 128).transpose(2, 1, 0))
    xb_, xr_ = _bsplit(xt)
    return np.ascontiguousarray(xb_), np.ascontiguousarray(xr_)


def kernel(x, q_proj, k_proj, v_proj, o_proj):
    if "nc" not in _CACHE:
        _CACHE["nc"] = build_nc()
    nc = _CACHE["nc"]

    xf_ = _x_planes(x)
    in_maps = [
        make_in_map(x, q_proj, k_proj, v_proj, o_proj, core, xf_=xf_)
        for core in range(N_CORES)
    ]

    try:
        res = run_bass_kernel_spmd(nc, in_maps, core_ids=list(range(N_CORES)))
    except Exception:
        # one retry: a fresh NRT session recovers transient device faults
        res = run_bass_kernel_spmd(nc, in_maps, core_ids=list(range(N_CORES)))
    _CACHE["last_results"] = res
    acc = np.zeros((N, D), dtype=np.float64)
    for core in range(N_CORES):
        acc += res.results[core]["out"].astype(np.float64)
    return acc.astype(np.float32)


if __name__ == "__main__":
    rng = np.random.default_rng(0)
    ins = {
        "x": rng.standard_normal((N, D), dtype=np.float32),
        "q_proj": rng.standard_normal((H, D, E), dtype=np.float32),
        "k_proj": rng.standard_normal((H, D, E), dtype=np.float32),
        "v_proj": rng.standard_normal((H, D, E), dtype=np.float32),
        "o_proj": rng.standard_normal((D, D), dtype=np.float32),
    }
    out = kernel(**ins)
    print("out", out.shape, out.dtype, np.abs(out).max())


# revision 82
# speedup vs baseline: 1.0016x; 1.0016x over previous
"""Multi-head attention (16 heads, N=2048, D=1024, E=64) on 8 Trainium2 cores.

Head-parallel sharding: core m handles heads (2m, 2m+1), computes its two
heads' attention contexts and a partial o_proj (rows 128m:128m+128 of the
row-sharded o_proj); the host sums the 8 partial fp32 outputs in fp64.

Numerics (validated baseline scheme): the softmax-score path is
fp32-accurate via an exact hi/lo float32r decomposition (host RNE-11
rounding == the PE's f32r operand rounding); v/ctx/o_proj tolerate plain
f32r.  Scores per [m=128, q=512] tile are two f32r matmuls: a stacked
K=128 cross-term matmul plus a K=65 hi matmul whose row 64 carries
-rowmax(q) (computed by a separate hi-only max pass in [q, m] layout,
DVE-reduced); exp((S-c)/8) on ACT; ctx^T/Z accumulate on PE via a
ones-column in v_ext; 1/Z broadcast+mul normalizes.

Schedule (v2, restructured):
  - x arrives HOST-split into hi/lo f32r planes [128, DCH, N] (c-major:
    per-c slices are 1KB-contiguous DMAs).  No device-side split work.
  - chunk 0 streams wq/xh as c-pair DMAs and defers the xl-dependent
    projection terms to the end of the accumulation group, so the first
    matmul issues ~4us earlier.
  - q/k hi/lo partition layouts alternate per head (and the k weight is
    head-swapped on host) so the psum->hi round-copy (ACT) and the lo
    subtract (DVE, psum+sbuf operands at equal start partitions) need no
    scratch; the ext-hi copy runs on Pool.  pt_q/pt_k psum tiles are
    double-buffered so the next chunk's matmuls never wait on unpack.
  - o_proj results are DMA'd DIRECTLY from PSUM to DRAM (no SBUF bounce).
  - the max-pass units ride the projection chunks (DVE is otherwise idle
    there): comp0 (m 0:1024) spreads over chunks 3-6, chunk 7 carries
    qc0's comp1 interleaved behind its k unpack, and attention chunks
    0-2 carry the remaining comp1 groups with late finish/staging.
  - the drain is pipelined: the last chunk runs its heads sequentially,
    the final normalize is emitted in 128-col slices, and the last
    o_proj blocks rotate through the score psum banks with per-half
    PSUM->DRAM DMAs on both HWDGE queues.
"""
import sys

sys.path.insert(0, "/opt/trn_rl_repo")

from contextlib import ExitStack

import ml_dtypes
import numpy as np

import concourse.bass as bass
import concourse.mybir as mybir
import concourse.tile as tile
from concourse import bacc
from concourse.bass_utils import run_bass_kernel_spmd
from concourse.masks import make_identity

# problem shapes (hardcoded per contract)
N = 2048
D = 1024
E = 64
H = 16
N_CORES = 8
H_PER_CORE = H // N_CORES  # 2

QC = 512          # q-chunk (moving dim of S'/ctx matmuls)
NQ = N // QC      # 4
MB = 128          # m-block (partition dim of S'^T tiles)
NMB = N // MB     # 16
DCH = D // 128    # 8 d-chunks for projections
PC = 256          # projection n-chunk
NPC = N // PC     # 8

F32 = mybir.dt.float32
F32R = mybir.dt.float32r
BF16 = mybir.dt.bfloat16

_CACHE = {}


def build_nc():
    nc = bacc.Bacc(None, target_bir_lowering=False, debug=False)

    # x^T as TWO bf16 planes (base + residual), c-major: together they
    # carry ~17 mantissa bits of x — bf16xbf16 products are EXACT in the
    # fp32 psum, so q/k keep full accuracy from 3 accumulation terms
    # while x moves HALF the bytes of fp32 and needs NO device split.
    xb = nc.declare_dram_parameter("xb", [128, DCH, N], BF16, isOutput=False)
    xr = nc.declare_dram_parameter("xr", [128, DCH, N], BF16, isOutput=False)
    # q/k weights arrive as bf16 base|residual pairs (cols 0:128 | 128:256);
    # k has the HEADS SWAPPED ([h1|h0]) so each head's hi rows land where
    # kx wants them; v is base-only.
    wq = nc.declare_dram_parameter("wq", [D, 256], BF16, isOutput=False)
    wk = nc.declare_dram_parameter("wk", [D, 256], BF16, isOutput=False)
    wv = nc.declare_dram_parameter("wv", [D, 128], BF16, isOutput=False)
    wo = nc.declare_dram_parameter("wo", [128, D], F32R, isOutput=False)
    # bf16 output: the host sums 8 partials in fp64, and bf16 rounding
    # of the partials costs ~2e-3 relative vs the 2e-2 budget — for HALF
    # the output DMA bytes (the drain flush is DMA-serialized)
    out = nc.declare_dram_parameter("out", [N, D], BF16, isOutput=True)

    with ExitStack() as ctx:
        tc = ctx.enter_context(tile.TileContext(nc))
        singles = ctx.enter_context(tc.tile_pool(name="singles", bufs=1))
        bc_pool = ctx.enter_context(tc.tile_pool(name="bc", bufs=2))

        ident = singles.tile([128, 128], F32)

        # long-lived SBUF tensors
        qT_ext = [singles.tile([65, N], F32R, tag=f"qT_ext{h}", name=f"qT_ext{h}")
                  for h in range(2)]
        kT_ext = [singles.tile([65, N], F32R, tag=f"kT_ext{h}", name=f"kT_ext{h}")
                  for h in range(2)]
        # stacked cross-term operands: one K=128 matmul computes
        # kl@qh + kh@ql.  Per-head partition layouts (hi where the psum
        # rows land, so unpack needs no partition shifts except the ext
        # copies):
        #   qx[0] = [qh; ql]   qx[1] = [ql; qh]
        #   kx[0] = [kl; kh]   kx[1] = [kh; kl]
        qx = [singles.tile([128, N], F32R, tag=f"qx{h}", name=f"qx{h}")
              for h in range(2)]
        kx = [singles.tile([128, N], F32R, tag=f"kx{h}", name=f"kx{h}")
              for h in range(2)]
        v_ext = [singles.tile([128, NMB, 65], F32R, tag=f"v_ext{h}",
                              name=f"v_ext{h}") for h in range(2)]
        mneg = [singles.tile([128, NQ], F32, tag=f"mneg{h}", name=f"mneg{h}")
                for h in range(2)]
        ctxn = singles.tile([128, N], F32R, tag="ctxn")
        wo_sb = singles.tile([128, D], F32R, tag="wo_sb")
        # per-chunk partial maxes: m4[qc][h][p, qbl, pair] (pair = 2 m-chunks)
        m4 = {}

        # hi partition ranges per head: qx hi at the head's own psum rows;
        # kx hi at the OTHER half (k weight is head-swapped on host)
        QHI = [slice(0, 64), slice(64, 128)]
        QLO = [slice(64, 128), slice(0, 64)]
        KHI = [slice(64, 128), slice(0, 64)]
        KLO = [slice(0, 64), slice(64, 128)]

        def mp_unit(pool, qc_t, qbl, mc, h, bufs=2, m0=None, mlen=QC,
                    comp=None):
            # one max-pass unit: one hi-only S[q, m] matmul (512-wide
            # m-chunk by default) into a 1-bank psum tile, one X-reduce
            # -> m4[qc_t][h][:, qbl, comp].
            qb = qc_t * (QC // 128) + qbl
            if m0 is None:
                m0 = mc * QC
            if comp is None:
                comp = mc
            t = pool.tile([128, QC], F32, tag="mp", name="mp", bufs=bufs)
            nc.tensor.matmul(
                t[:, 0:mlen],
                qT_ext[h][0:64, qb * 128:(qb + 1) * 128],
                kT_ext[h][0:64, m0:m0 + mlen],
                start=True,
                stop=True,
            )
            nc.vector.tensor_reduce(
                out=m4[qc_t][h][:, qbl, comp:comp + 1],
                in_=t[:, 0:mlen],
                axis=mybir.AxisListType.X, op=mybir.AluOpType.max,
            )

        # per-generation partial-max tiles (tiny; one per (gen, head) so
        # no pool-rotation hazard can serialize a late finish against an
        # early next-generation unit)
        for g in range(NQ):
            nco = 5 if g == 0 else NQ
            m4[g] = [singles.tile([128, NQ, nco], F32, tag=f"m4_{g}_{h}",
                                  name=f"m4_{g}_{h}") for h in range(2)]

        def mp_finish_reduce(qc):
            for h in range(2):
                # combine the component maxes per q-block, negated
                nc.vector.tensor_reduce(
                    out=mneg[h], in_=m4[qc][h],
                    axis=mybir.AxisListType.X, op=mybir.AluOpType.max,
                    negate=True,
                )

        def mp_finish_stage(qc, ptm_pool, ptm_tag, ptm_shape):
            qsl = slice(qc * QC, (qc + 1) * QC)
            # stage -max into qT_ext row 64: transpose [128, 4] -> [4, 128]
            # (rounded to f32r); the partition-major stream of [4, 128] is
            # exactly [1, 512].
            for h in range(2):
                ptm = ptm_pool.tile(ptm_shape, F32, tag=ptm_tag, name="ptm",
                                    space="PSUM")
                nc.tensor.transpose(ptm[0:4, 0:128], mneg[h], ident)
                mt_sb = bc_pool.tile([4, 128], F32R, tag="mt_sb")
                # ACT copy: keeps the staging chain off the DVE queue
                nc.scalar.copy(out=mt_sb, in_=ptm[0:4, 0:128])
                nc.sync.dma_start(out=qT_ext[h][64:65, qsl], in_=mt_sb)

        # ---------------- phase 1: projections ----------------
        with tc.tile_pool(name="mp1", bufs=2, space="PSUM") as mp1, \
             tc.tile_pool(name="xs", bufs=3) as xs_pool, \
             tc.tile_pool(name="ph1", bufs=1) as ph1:
            ones_cols = ph1.tile([128, NMB, 1], F32)
            ones_row = ph1.tile([1, N], F32)

            w_sb = {
                "q": ph1.tile([128, DCH, 256], BF16, tag="w_q", name="w_q"),
                "k": ph1.tile([128, DCH, 256], BF16, tag="w_k", name="w_k"),
                "v": ph1.tile([128, DCH, 128], BF16, tag="w_v", name="w_v"),
            }
            wq_r = wq.rearrange("(c p) e -> p c e", p=128)
            wk_r = wk.rearrange("(c p) e -> p c e", p=128)
            wv_r = wv.rearrange("(c p) e -> p c e", p=128)

            # max-pass ride schedule: unit (g, qbl, mc, h) is ready at the
            # end of proj chunk max(qb//2, 2mc+1) (q/k unpack of its
            # operand ranges).  mc=3 units depend on chunk 7's k unpack:
            # qc0's interleave chunk 7's q matmuls, the rest run in
            # attention.  Greedy placement in generation order fills the
            # proj chunks up to a DVE-budget capacity; the remainder
            # streams through attention slots with per-generation
            # finish/staging emitted right after the last unit.
            proj_cap = {1: 4, 2: 5, 3: 8, 4: 8, 5: 8, 6: 3, 7: 0}
            ride = {c: [] for c in proj_cap}
            att_rest = []
            units_012 = [(g, qbl, mc, h)
                         for g in range(4) for mc in range(3)
                         for qbl in range(4) for h in range(2)]
            for u in sorted(units_012,
                            key=lambda u: (u[0], u[2], u[1], u[3])):
                g, qbl, mc, h = u
                r = max((g * 4 + qbl) // 2, 2 * mc + 1)
                for c in range(max(r, 1), NPC):
                    if len(ride[c]) < proj_cap[c]:
                        ride[c].append(u)
                        break
                else:
                    att_rest.append(u)
            # attention stream: gen1 mc3 first (earliest deadline), then
            # per-generation leftovers + mc3 units
            att_stream = []
            for g in range(1, 4):
                att_stream += [(g, qbl, 3, h)
                               for qbl in range(4) for h in range(2)]
                att_stream += [u for u in att_rest if u[0] == g]
            # sanity: nothing from gen0 may remain
            assert not [u for u in att_rest if u[0] == 0]
            # assign to (chunk, t) slots, inserting finish/staging after
            # each generation completes; staging(g) must land in an
            # attention chunk < g.  Chunk 0 only takes even slots (its
            # entry also absorbs the post-proj DVE backlog).
            # chunk 0 takes sparse slots starting at t=6 (its entry also
            # absorbs the chunk-7/post-proj DVE backlog), denser later;
            # chunks 1-2 dense
            slot_list = ([(0, t) for t in range(6, 17, 2)]
                         + [(0, t) for t in range(17, 32)]
                         + [(1, t) for t in range(32)]
                         + [(2, t) for t in range(32)])
            att_mp = {0: {}, 1: {}, 2: {}}
            att_fin = {0: {}, 1: {}, 2: {}}
            slot = 0
            for g in range(1, 4):
                gu = [u for u in att_stream if u[0] == g]
                for u in gu:
                    fc, ft = slot_list[slot]
                    att_mp.setdefault(fc, {}).setdefault(ft, []).append(u)
                    slot += 1
                fc, ft = slot_list[slot]
                assert fc < g, f"gen {g} staging lands in chunk {fc}"
                att_fin.setdefault(fc, {}).setdefault(ft, []).append(g)

            xtiles = {}

            def xtile(c):
                if c not in xtiles:
                    xtiles[c] = (
                        xs_pool.tile([128, DCH, PC], BF16, tag="xbt",
                                     name=f"xbt{c}", bufs=3),
                        xs_pool.tile([128, DCH, PC], BF16, tag="xrt",
                                     name=f"xrt{c}", bufs=3),
                    )
                return xtiles[c]

            def xsl(c):
                return slice(c * PC, (c + 1) * PC)

            with tc.tile_pool(name="pp", bufs=1, space="PSUM") as pp:
                for nchunk in range(NPC):
                    sl = xsl(nchunk)
                    xbt, xrt = xtile(nchunk)
                    if nchunk == 0:
                        # c-pair streaming on a single queue: the first q
                        # matmuls start once wq01+xb01 land (~3.6us);
                        # wk/xr/xb1/wv queue behind.  The xr-dependent
                        # terms are emitted last in each accumulation
                        # group, so only the base plane gates the start.
                        for cs in (slice(0, 2), slice(2, 4), slice(4, 8)):
                            nc.sync.dma_start(out=w_sb["q"][:, cs, :],
                                              in_=wq_r[:, cs, :])
                            nc.sync.dma_start(out=xbt[:, cs, :],
                                              in_=xb[:, cs, sl])
                        nc.sync.dma_start(out=w_sb["k"], in_=wk_r)
                        nc.sync.dma_start(out=xrt, in_=xr[:, :, sl])
                        nxb, nxr = xtile(1)
                        nc.sync.dma_start(out=nxb, in_=xb[:, :, xsl(1)])
                        nc.sync.dma_start(out=w_sb["v"], in_=wv_r)
                        nc.sync.dma_start(out=nxr, in_=xr[:, :, xsl(1)])
                        # constants built here, off the critical queues
                        # (ident feeds chunk 0's v_ext transposes)
                        make_identity(nc, ident)
                        # PE warm-up: the p-state ramp needs ~3us of
                        # continuous execution to reach 2.4GHz; dummy
                        # transposes burn the DMA-latency window so the
                        # first real matmuls run at full speed
                        for wu in range(12):
                            wut = pp.tile([128, PC], F32, tag="pt_v",
                                          name="warm", bufs=2)
                            nc.tensor.transpose(wut[:, 0:128], ident, ident)
                    elif nchunk < NPC - 1:
                        # prefetch the NEXT chunk's x planes
                        nxb, nxr = xtile(nchunk + 1)
                        nc.sync.dma_start(out=nxb,
                                          in_=xb[:, :, xsl(nchunk + 1)])
                        nc.sync.dma_start(out=nxr,
                                          in_=xr[:, :, xsl(nchunk + 1)])
                        if nchunk == 3:
                            # wo is first read at attention qc1 (~85us);
                            # on the SP queue so it cannot be hoisted into
                            # the startup DMA wall
                            nc.sync.dma_start(out=wo_sb, in_=wo[:, :])
                    if nchunk == 1:
                        # ones constants: built here so their memsets can't
                        # be scheduler-hoisted ahead of make_identity on
                        # the Pool queue (the warm-up chain needs ident)
                        nc.gpsimd.memset(ones_cols, 1.0)
                        nc.gpsimd.memset(ones_row, 1.0)
                    if nchunk in (2, 3):
                        # extension constants (kT_ext row 64 = 1, v_ext
                        # col 64 = 1): written mid-proj where ACT has
                        # slack — attention(0)'s first ext/ctx matmuls
                        # read them immediately at the phase boundary
                        h = nchunk - 2
                        nc.scalar.copy(out=kT_ext[h][64:65, :], in_=ones_row)
                        nc.scalar.copy(out=v_ext[h][:, :, 64:65],
                                       in_=ones_cols)
                    ride_units = ride.get(nchunk, [])
                    pt = {
                        name: pp.tile([128, PC], F32, tag=f"pt_{name}",
                                      name=f"pt_{name}", bufs=2)
                        for name in ("q", "k", "v")
                    }

                    def emit8(name, wsl, xt_, start, stop):
                        for c in range(DCH):
                            nc.tensor.matmul(
                                pt[name],
                                w_sb[name][:, c, wsl],
                                xt_[:, c, :],
                                start=(start and c == 0),
                                stop=(stop and c == DCH - 1),
                            )

                    def unpack(name):
                        dst_ext = qT_ext if name == "q" else kT_ext
                        dst_x = qx if name == "q" else kx
                        hi_sl = QHI if name == "q" else KHI
                        lo_sl = QLO if name == "q" else KLO
                        # unpack: ACT round-copies psum -> f32r hi (same
                        # partitions); DVE subtracts (psum, sbuf-hi) ->
                        # f32r lo at the opposite half; Pool copies the
                        # ext hi view (partition shift where needed).
                        for h in range(2):
                            # psum rows for this head: q keeps [h0|h1],
                            # k is head-swapped on host so hs == hi_sl[h]
                            hs = hi_sl[h] if name == "k" else \
                                slice(h * 64, (h + 1) * 64)
                            nc.scalar.copy(
                                out=dst_x[h][hi_sl[h], sl],
                                in_=pt[name][hs, :])
                            nc.vector.tensor_sub(
                                dst_x[h][lo_sl[h], sl],
                                pt[name][hs, :], dst_x[h][hi_sl[h], sl])
                            if hi_sl[h].start == 0:
                                # partition-aligned: Pool takes it
                                nc.gpsimd.tensor_copy(
                                    dst_ext[h][0:64, sl],
                                    dst_x[h][hi_sl[h], sl])
                            else:
                                # partition SHIFT (64:128 -> 0:64): only
                                # ACT is known to support this
                                nc.scalar.copy(
                                    out=dst_ext[h][0:64, sl],
                                    in_=dst_x[h][hi_sl[h], sl])

                    def emit_v():
                        emit8("v", slice(0, 128), xbt, True, True)
                        vT_c = xs_pool.tile([128, PC], F32, tag="vT_c")
                        nc.scalar.copy(out=vT_c, in_=pt["v"])
                        # v_ext: transpose vT [64, 128-block] -> v [m, e]
                        # blocks [128, 64], inline per chunk
                        for nb2 in range(PC // 128):
                            mb = nchunk * (PC // 128) + nb2
                            for h in range(2):
                                ptt = pp.tile([128, PC], F32, tag="pt_v",
                                              name="ptt", bufs=2)
                                nc.tensor.transpose(
                                    ptt[:, 0:64],
                                    vT_c[h * 64:(h + 1) * 64,
                                         nb2 * 128:(nb2 + 1) * 128],
                                    ident[h * 64:(h + 1) * 64,
                                          h * 64:(h + 1) * 64],
                                )
                                nc.scalar.copy(out=v_ext[h][:, mb, 0:64],
                                               in_=ptt[:, 0:64])

                    HIW, LOW = slice(0, 128), slice(128, 256)
                    if nchunk == 0:
                        # q runs all three terms first (wq is first on the
                        # queue); k/v follow as their weights land
                        emit8("q", HIW, xbt, True, False)
                        emit8("q", LOW, xbt, False, False)
                        emit8("q", HIW, xrt, False, True)
                        unpack("q")
                        emit8("k", HIW, xbt, True, False)
                        emit8("k", LOW, xbt, False, False)
                        emit8("k", HIW, xrt, False, True)
                        unpack("k")
                        emit_v()
                    elif nchunk < NPC - 1:
                        # base-plane terms first, residual terms after,
                        # v last; rides LAST (their operands need this
                        # chunk's unpack)
                        emit8("q", HIW, xbt, True, False)
                        emit8("k", HIW, xbt, True, False)
                        emit8("q", LOW, xbt, False, False)
                        emit8("k", LOW, xbt, False, False)
                        emit8("q", HIW, xrt, False, True)
                        unpack("q")
                        emit8("k", HIW, xrt, False, True)
                        unpack("k")
                        emit_v()
                        for u in ride_units:
                            mp_unit(mp1, *u)
                        if nchunk == NPC - 2:
                            # gen0's mc3a (m 1536:1792) is ready here —
                            # only its second half must wait for chunk 7
                            for u in range(8):
                                mp_unit(mp1, 0, u // 2, 3, u % 2,
                                        m0=1536, mlen=256, comp=3)
                    else:
                        # chunk 7: k completes first; gen0's mc3b units
                        # (m 1792:2048, the only slice still missing)
                        # fire right behind its unpack so the staging
                        # chain completes before attention needs it
                        emit8("k", HIW, xbt, True, False)
                        emit8("k", LOW, xbt, False, False)
                        emit8("k", HIW, xrt, False, True)
                        unpack("k")
                        emit8("q", HIW, xbt, True, False)
                        for u in range(8):
                            mp_unit(mp1, 0, u // 2, 3, u % 2,
                                    m0=1792, mlen=256, comp=4)
                        # gen0's finish + staging fire as soon as its last
                        # unit reduces — ahead of this chunk's q unpack on
                        # the DVE queue, so attention(0) never waits
                        mp_finish_reduce(0)
                        mp_finish_stage(0, mp1, "mp", [128, QC])
                        emit8("q", LOW, xbt, False, False)
                        emit_v()
                        for u in ride_units:
                            mp_unit(mp1, *u)
                        emit8("q", HIW, xrt, False, True)
                        unpack("q")


        # ---------------- phase 2: attention chunks ----------------
        ex_pool = ctx.enter_context(tc.tile_pool(name="ex", bufs=4))
        sp_ps = ctx.enter_context(tc.tile_pool(name="sp", bufs=3, space="PSUM"))
        ctx_pool = ctx.enter_context(tc.tile_pool(name="cx", bufs=1, space="PSUM"))
        mp2 = ctx.enter_context(tc.tile_pool(name="mp2", bufs=1, space="PSUM"))

        def norm_head(qc, h, ctx_ps, sliced=False):
            qsl = slice(qc * QC, (qc + 1) * QC)
            hrows = slice(h * 64, (h + 1) * 64)
            rz = bc_pool.tile([1, QC], F32, tag="rz")
            bc_sb = bc_pool.tile([64, QC], F32, tag="bc_sb")
            if not sliced:
                nc.vector.reciprocal(out=rz, in_=ctx_ps[h][64:65, :])
                nc.gpsimd.partition_broadcast(bc_sb, rz)
                nc.vector.tensor_mul(
                    ctxn[hrows, qsl], ctx_ps[h][0:64, :], bc_sb
                )
                return
            # final-chunk drain: 128-col recip/broadcast/mul chains so the
            # first o_proj block starts ~1us earlier
            for s4 in range(4):
                ssl = slice(s4 * 128, (s4 + 1) * 128)
                nc.vector.reciprocal(out=rz[:, ssl],
                                     in_=ctx_ps[h][64:65, ssl])
                nc.gpsimd.partition_broadcast(bc_sb[:, ssl], rz[:, ssl])
                nc.vector.tensor_mul(
                    ctxn[hrows, qc * QC + s4 * 128:qc * QC + (s4 + 1) * 128],
                    ctx_ps[h][0:64, ssl], bc_sb[:, ssl]
                )

        def oproj_block(qc, nb, fine_dma=False):
            # o_proj for one 128-row n-block (both heads fused: K=128);
            # the two 512-wide psum halves merge into one [128, 1024]
            # SBUF tile so the block is a single contiguous output DMA.
            # The psum->SBUF copies split across DVE (dc=0) and ACT
            # (dc=1) to balance both engines' attention-phase load.
            n0 = qc * QC + nb * 128
            if fine_dma:
                # drain path: no score matmuls left, so the sp 3-bank
                # rotation pipelines the final o_proj matmuls; per-half
                # SBUF tiles (4-deep) and output DMAs alternating between
                # the two HWDGE queues (SP/ACT) keep the drain flowing
                # both 512-halves merge into one [128, 1024] SBUF tile:
                # with bf16 the 128KB half-transfers are shorter than the
                # 625ns HWDGE issue slots, so ONE DMA per block halves
                # the issue-bound flush
                pf = ex_pool.tile([128, D], BF16, tag="po_f2", bufs=4)
                for dc in range(D // QC):
                    pool_, tag_ = (sp_ps, "sp") if (2 * nb + dc) % 2 == 0 \
                        else (mp2, "mp")
                    po = pool_.tile([128, QC], F32, tag=tag_, name="po",
                                    bufs=3)
                    nc.tensor.matmul(
                        po,
                        ctxn[:, n0:n0 + 128],
                        wo_sb[:, dc * QC:(dc + 1) * QC],
                        start=True,
                        stop=True,
                    )
                    # first half of the flush copies on ACT (DVE still
                    # owns the norm chain), second half on DVE
                    dst = pf[:, dc * QC:(dc + 1) * QC]
                    if nb <= 1:
                        nc.scalar.copy(out=dst, in_=po)
                    else:
                        nc.vector.tensor_copy(dst, po)
                dma_eng = nc.sync if nb % 2 == 0 else nc.scalar
                dma_eng.dma_start(out=out[n0:n0 + 128, :], in_=pf)
                return
            po_sb = ex_pool.tile([128, D], BF16, tag="po_sb", bufs=2)
            for dc in range(D // QC):
                # share the attention "mp" 3-deep psum rotation (frees a
                # dedicated bank and absorbs DVE jitter)
                po = mp2.tile([128, QC], F32, tag="mp", name="po", bufs=3)
                nc.tensor.matmul(
                    po,
                    ctxn[:, n0:n0 + 128],
                    wo_sb[:, dc * QC:(dc + 1) * QC],
                    start=True,
                    stop=True,
                )
                dst = po_sb[:, dc * QC:(dc + 1) * QC]
                if dc == 1 and qc != 2:
                    nc.scalar.copy(out=dst, in_=po)
                else:
                    nc.vector.tensor_copy(dst, po)
            nc.sync.dma_start(out=out[n0:n0 + 128, :], in_=po_sb)

        def attention_chunk(qc, seq_heads=False):
            qsl = slice(qc * QC, (qc + 1) * QC)
            ctx_ps = [ctx_pool.tile([65, QC], F32, tag=f"ctx{h}",
                                    name=f"ctx_ps{h}") for h in range(2)]
            if not seq_heads:
                # h0's ctx matmuls lead (h1's previous-chunk norm frees
                # its bank meanwhile), and h0 finishes ~10 tiles early so
                # its norm chain runs under h1's tail
                heads_order = ([(mb, 0) for mb in range(6)]
                               + [(mb, 1) for mb in range(6)]
                               + [(mb, 0) for mb in range(6, NMB)]
                               + [(mb, 1) for mb in range(6, NMB)])
            else:
                heads_order = [(mb, h) for h in range(2) for mb in range(NMB)]
            started = {0: False, 1: False}

            # interleave schedules: precomputed att_mp/att_fin slot maps
            mp_sched = att_mp.get(qc, {})
            fin_sched = att_fin.get(qc, {})
            po_sched = {}
            if qc > 0:
                for nb in range(4):
                    po_sched[11 + 4 * nb] = (qc - 1, nb)

            def emit_m1_tail(sp, mb, h):
                # the only matmul that reads row 64 (the staged -max row);
                # lagging it one tile behind the cross matmul hides the
                # staging DMA latency at chunk entry
                nc.tensor.matmul(
                    sp, kT_ext[h][:, mb * 128:(mb + 1) * 128],
                    qT_ext[h][:, qsl],
                    start=False, stop=True,
                )
                et = ex_pool.tile([128, QC], F32R, tag="et", name="et")
                nc.scalar.activation(
                    out=et, in_=sp,
                    func=mybir.ActivationFunctionType.Exp, scale=0.125,
                )
                nc.tensor.matmul(
                    ctx_ps[h], v_ext[h][:, mb, :], et,
                    start=not started[h], stop=(mb == NMB - 1),
                )
                started[h] = True
                if mb == NMB - 1 and not seq_heads:
                    # normalize as soon as this head's accumulation closes
                    norm_head(qc, h, ctx_ps)

            lagged = []
            for it, (mb, h) in enumerate(heads_order):
                for g in fin_sched.get(it, ()):
                    mp_finish_reduce(g)
                    mp_finish_stage(g, sp_ps, "sp", [128, QC])
                for u in mp_sched.get(it, ()):
                    mp_unit(mp2, *u, bufs=3)
                if it in po_sched:
                    pqc, pnb = po_sched[it]
                    oproj_block(pqc, pnb)
                msl = slice(mb * 128, (mb + 1) * 128)
                sp = sp_ps.tile([128, QC], F32, tag="sp", name=f"sp{h}")
                # stacked cross terms first (no row-64 dependency):
                # one K=128 matmul = kl@qh + kh@ql
                nc.tensor.matmul(
                    sp, kx[h][:, msl], qx[h][:, qsl],
                    start=True, stop=False,
                )
                lagged.append((sp, mb, h))
                if len(lagged) > 2:
                    emit_m1_tail(*lagged.pop(0))
                if seq_heads and mb == NMB - 1:
                    while lagged:
                        emit_m1_tail(*lagged.pop(0))
                    norm_head(qc, h, ctx_ps, sliced=(h == 1))
            while lagged:
                emit_m1_tail(*lagged.pop(0))
            return ctx_ps

        for qc in range(NQ):
            seq = qc == NQ - 1
            attention_chunk(qc, seq_heads=seq)
        for nb in range(4):
            oproj_block(NQ - 1, nb, fine_dma=True)

    nc.compile()
    return nc


def _round11(x):
    # round-to-nearest-even to 11 explicit mantissa bits — exactly the
    # hardware's float32r operand rounding (verified on device)
    u = np.ascontiguousarray(x, dtype=np.float32).view(np.uint32)
    shift = 23 - 11
    add = np.uint32((1 << (shift - 1)) - 1)
    lsb = (u >> np.uint32(shift)) & np.uint32(1)
    mask = np.uint32(~((1 << shift) - 1) & 0xFFFFFFFF)
    return ((u + add + lsb) & mask).view(np.float32)


def _split11(x):
    hi = _round11(x)
    lo = _round11(x.astype(np.float32) - hi)
    return hi, lo


def _bsplit(a):
    # bf16 base + bf16 residual: ~17 mantissa bits total
    b = a.astype(ml_dtypes.bfloat16)
    r = (a.astype(np.float32) - b.astype(np.float32)).astype(ml_dtypes.bfloat16)
    return b, r


def make_in_map(x, q_proj, k_proj, v_proj, o_proj, core, xf_=None):
    h0 = core * H_PER_CORE
    if xf_ is None:
        xf_ = _x_planes(x)
    xb_, xr_ = xf_

    def wcat2(w, swap):
        pair = [w[h0 + 1], w[h0]] if swap else [w[h0], w[h0 + 1]]
        w2 = np.concatenate(pair, axis=1).astype(np.float32)
        wb, wr = _bsplit(w2)
        return np.ascontiguousarray(np.concatenate([wb, wr], axis=1))

    def wcat(w, swap):
        pair = [w[h0 + 1], w[h0]] if swap else [w[h0], w[h0 + 1]]
        return np.ascontiguousarray(
            np.concatenate(pair, axis=1).astype(np.float32))

    return {
        "xb": xb_,
        "xr": xr_,
        "wq": wcat2(q_proj, False),
        "wk": wcat2(k_proj, True),
        "wv": wcat(v_proj, False).astype(ml_dtypes.bfloat16),
        "wo": _round11(o_proj[h0 * 64:(h0 + 2) * 64, :]),
    }


def _x_planes(x):
    # x^T c-major: [p, c, n] = x[n, 128c+p], split into bf16 base+residual
    xt = np.ascontiguousarray(
        x.astype(np.float32, copy=False).reshape(N, DCH, 128).transpose(2, 1, 0))
    xb_, xr_ = _bsplit(xt)
    return np.ascontiguousarray(xb_), np.ascontiguousarray(xr_)


def kernel(x, q_proj, k_proj, v_proj, o_proj):
    if "nc" not in _CACHE:
        _CACHE["nc"] = build_nc()
    nc = _CACHE["nc"]

    xf_ = _x_planes(x)
    in_maps = [
        make_in_map(x, q_proj, k_proj, v_proj, o_proj, core, xf_=xf_)
        for core in range(N_CORES)
    ]

    try:
        res = run_bass_kernel_spmd(nc, in_maps, core_ids=list(range(N_CORES)))
    except Exception:
        # one retry: a fresh NRT session recovers transient device faults
        res = run_bass_kernel_spmd(nc, in_maps, core_ids=list(range(N_CORES)))
    _CACHE["last_results"] = res
    acc = np.zeros((N, D), dtype=np.float64)
    for core in range(N_CORES):
        acc += res.results[core]["out"].astype(np.float64)
    return acc.astype(np.float32)


if __name__ == "__main__":
    rng = np.random.default_rng(0)
    ins = {
        "x": rng.standard_normal((N, D), dtype=np.float32),
        "q_proj": rng.standard_normal((H, D, E), dtype=np.float32),
        "k_proj": rng.standard_normal((H, D, E), dtype=np.float32),
        "v_proj": rng.standard_normal((H, D, E), dtype=np.float32),
        "o_proj": rng.standard_normal((D, D), dtype=np.float32),
    }
    out = kernel(**ins)
    print("out", out.shape, out.dtype, np.abs(out).max())
